# revision 1
# baseline (speedup 1.0000x reference)
"""Trainium2 Bass kernel for nn_MultiHeadCulturalAttention.

Sharding (8 cores, SPMD single program with a partition-id branch):
  cores 0-3: "regular" branch — (batch b = core//2), 3 heads of hd=128 each
  cores 4-7: "cultural" branch — (batch b = (core-4)//2), 1 head of hd=384

Every core computes Q/K/V projections for its 384 feature columns, its
attention maps (fully transposed dataflow: Q^T/K^T produced directly by
weight-stationary projections; scores^T computed per s-tile so the
attention-mask add and softmax scale fold into the Exp activation bias;
AV with V-stationary matmuls produces out^T which feeds the folded
output projection wfold = branch_out_w @ out_w_half), then DMAs a
[2048, 768] partial of the final output. The host sums 4 partials per
batch and adds a precomputed constant bias vector.

All matmul operands are float32r (full-rate on the PE at N>=256);
accumulation stays fp32 in PSUM.
"""
import numpy as np

import concourse.bass as bass
import concourse.mybir as mybir
from concourse import bacc
from concourse.tile import TileContext
from concourse.bass_utils import run_bass_kernel_spmd

F32 = mybir.dt.float32
F32R = mybir.dt.float32r
AF = mybir.ActivationFunctionType
ALU = mybir.AluOpType

B, T, E = 2, 2048, 768
NE = E // 128            # 6 e-chunks
NT = T // 128            # 16 tiles along seq
F = 384                  # per-core projection width (3 reg heads / 1 cul head)
NF = F // 128            # 3 f-tiles
SCALE_REG = float(128 ** -0.5)
SCALE_CUL = float(384 ** -0.5)

_NC_CACHE = None


def _proj_transposed(nc, pool_ps, pool_out, sb_x, sb_w, sb_bias, tag):
    """Q^T/K^T: out[f,t] = sum_e w[e,f] x^T[e,t], evacuated with per-partition
    bias add. Returns 3 SBUF tiles [128, T] (f32r), one per f-tile."""
    outs = []
    for f in range(NF):
        tiles_th = []
        for th in range(2):  # halves of T -> psum [128, 1024]
            ps = pool_ps.tile([128, 1024], F32, tag="pp")
            for e in range(NE):
                for tq in range(2):
                    nc.tensor.matmul(
                        ps[:, tq * 512:(tq + 1) * 512],
                        lhsT=sb_w[:, e * F + f * 128: e * F + (f + 1) * 128],
                        rhs=sb_x[:, e * T + th * 1024 + tq * 512: e * T + th * 1024 + (tq + 1) * 512],
                        start=(e == 0), stop=(e == NE - 1))
            tiles_th.append(ps)
        sb_o = pool_out.tile([128, T], F32R, tag=f"{tag}{f}")
        for th in range(2):
            nc.scalar.activation(sb_o[:, th * 1024:(th + 1) * 1024], tiles_th[th][:],
                                 AF.Identity, bias=sb_bias[:, f:f + 1], scale=1.0)
        outs.append(sb_o)
    return outs


def _proj_v(nc, pool_ps, pool_out, sb_x, sb_wv):
    """V: out[s,d] = sum_e x^T[e,s] wv[e,d]. Returns 16 tiles [128, F] f32r."""
    outs = []
    for s in range(NT):
        ps = pool_ps.tile([128, F], F32, tag="ppv")
        for e in range(NE):
            nc.tensor.matmul(
                ps[:], lhsT=sb_x[:, e * T + s * 128: e * T + (s + 1) * 128],
                rhs=sb_wv[:, e * F:(e + 1) * F],
                start=(e == 0), stop=(e == NE - 1))
        sb_v = pool_out.tile([128, F], F32R, tag=f"v{s}")
        nc.vector.tensor_copy(sb_v[:], ps[:])
        outs.append(sb_v)
    return outs


def _attention(nc, tc, sb_q, sb_k, sb_v, sb_attn, sb_ones, outT,
               maps, t_win, scale, cmask=None, psc_bufs=2):
    """maps: list of (score_chunk_list, av_list) where av_list is
    [(v_col_chunk, out_tile_idx), ...]. t_win: t window width."""
    n_ti = T // t_win
    n_tq = t_win // 512
    stg_w = t_win // 128
    from contextlib import ExitStack
    stk = ExitStack()
    with stk:
        pools = {
            "psc": stk.enter_context(tc.tile_pool(name="psc", bufs=psc_bufs, space="PSUM")),
            "po": stk.enter_context(tc.tile_pool(name="po", bufs=1, space="PSUM")),
            "psum_sum": stk.enter_context(tc.tile_pool(name="psum_sum", bufs=1, space="PSUM")),
            "wt": stk.enter_context(tc.tile_pool(name="wt", bufs=3)),
            "cm": stk.enter_context(tc.tile_pool(name="cm", bufs=3)),
            "srow": stk.enter_context(tc.tile_pool(name="srow", bufs=2)),
        }
        _attention_body(nc, tc, pools, sb_q, sb_k, sb_v, sb_attn, sb_ones, outT,
                        maps, t_win, scale, cmask, n_ti, n_tq, stg_w)


def _attention_body(nc, tc, pools, sb_q, sb_k, sb_v, sb_attn, sb_ones, outT,
                    maps, t_win, scale, cmask, n_ti, n_tq, stg_w):
    for chunks, avs in maps:
        for ti in range(n_ti):
            t0 = ti * t_win
            ps_o = [pools["po"].tile([128, t_win], F32, tag=f"po{j}", name=f"ps_o{j}")
                    for j, _ in enumerate(avs)]
            ps_sum = pools["psum_sum"].tile([1, t_win], F32, tag="psum_sum")
            for s in range(NT):
                ps_sc = pools["psc"].tile([128, t_win], F32, tag="psc")
                for ci, c in enumerate(chunks):
                    for tq in range(n_tq):
                        nc.tensor.matmul(
                            ps_sc[:, tq * 512:(tq + 1) * 512],
                            lhsT=sb_k[c][:, s * 128:(s + 1) * 128],
                            rhs=sb_q[c][:, t0 + tq * 512: t0 + (tq + 1) * 512],
                            start=(ci == 0), stop=(ci == len(chunks) - 1))
                if cmask is not None:
                    cm = pools["cm"].tile([128, t_win], F32, tag="cm")
                    nc.sync.dma_start(
                        out=cm[:], in_=cmask[s * 128:(s + 1) * 128, t0:t0 + t_win])
                    nc.vector.tensor_tensor(ps_sc[:], ps_sc[:], cm[:], ALU.add)
                wt = pools["wt"].tile([128, t_win], F32R, tag="wt")
                nc.scalar.activation(wt[:], ps_sc[:], AF.Exp,
                                     bias=sb_attn[:, s:s + 1], scale=scale)
                for j, (vc, _oidx) in enumerate(avs):
                    for tq in range(n_tq):
                        nc.tensor.matmul(
                            ps_o[j][:, tq * 512:(tq + 1) * 512],
                            lhsT=sb_v[s][:, vc * 128:(vc + 1) * 128],
                            rhs=wt[:, tq * 512:(tq + 1) * 512],
                            start=(s == 0), stop=(s == NT - 1))
                for tq in range(n_tq):
                    nc.tensor.matmul(
                        ps_sum[0:1, tq * 512:(tq + 1) * 512],
                        lhsT=sb_ones[:], rhs=wt[:, tq * 512:(tq + 1) * 512],
                        start=(s == 0), stop=(s == NT - 1))
            # normalize: rec = 1/sum broadcast over partitions, mult into outT
            sum_row = pools["srow"].tile([1, t_win], F32, tag="srow")
            nc.vector.tensor_copy(sum_row[:], ps_sum[:])
            stg = pools["srow"].tile([128, stg_w], F32, tag="stg")
            nc.sync.dma_start(out=stg[:], in_=sum_row[:])
            rec = pools["srow"].tile([128, stg_w], F32, tag="rec")
            nc.vector.reciprocal(rec[:], stg[:])
            rec_row = pools["srow"].tile([1, t_win], F32, tag="recrow")
            nc.sync.dma_start(out=rec_row[:], in_=rec[:])
            rec_b = pools["srow"].tile([128, t_win], F32, tag="recb")
            nc.gpsimd.partition_broadcast(rec_b[:], rec_row[:])
            for j, (_vc, oidx) in enumerate(avs):
                nc.vector.tensor_tensor(outT[oidx][:, t0:t0 + t_win],
                                        ps_o[j][:], rec_b[:], ALU.mult)


def _build_nc():
    nc = bacc.Bacc()
    d_xT = nc.declare_dram_parameter("xT", [128, NE * T], F32R, isOutput=False)
    d_wq = nc.declare_dram_parameter("wq", [128, NE * F], F32R, isOutput=False)
    d_wk = nc.declare_dram_parameter("wk", [128, NE * F], F32R, isOutput=False)
    d_wv = nc.declare_dram_parameter("wv", [128, NE * F], F32R, isOutput=False)
    d_qb = nc.declare_dram_parameter("qb", [128, NF], F32, isOutput=False)
    d_kb = nc.declare_dram_parameter("kb", [128, NF], F32, isOutput=False)
    d_attn = nc.declare_dram_parameter("attn", [128, NT], F32, isOutput=False)
    d_wfold = nc.declare_dram_parameter("wfold", [128, NF * E], F32R, isOutput=False)
    d_ones = nc.declare_dram_parameter("ones", [128, 1], F32R, isOutput=False)
    d_cmask = nc.declare_dram_parameter("cmask", [T, T], F32, isOutput=False)
    d_out = nc.declare_dram_parameter("out", [T, E], F32, isOutput=True)

    with TileContext(nc) as tc:
        pid = nc.partition_id()
        from contextlib import ExitStack
        with ExitStack() as stk:
            # ---- persistent pools (live through maps + fold) ----
            p_small = stk.enter_context(tc.tile_pool(name="small", bufs=1))
            p_qt = stk.enter_context(tc.tile_pool(name="qt", bufs=1))
            p_kt = stk.enter_context(tc.tile_pool(name="kt", bufs=1))
            p_v = stk.enter_context(tc.tile_pool(name="vp", bufs=1))
            p_outT = stk.enter_context(tc.tile_pool(name="outT", bufs=1))
            p_wfold = stk.enter_context(tc.tile_pool(name="wfp", bufs=1))

            sb_qb = p_small.tile([128, NF], F32)
            sb_kb = p_small.tile([128, NF], F32)
            sb_attn = p_small.tile([128, NT], F32)
            sb_ones = p_small.tile([128, 1], F32R)
            sb_wfold = p_wfold.tile([128, NF * E], F32R)
            nc.sync.dma_start(out=sb_qb[:], in_=d_qb[:])
            nc.sync.dma_start(out=sb_kb[:], in_=d_kb[:])
            nc.sync.dma_start(out=sb_attn[:], in_=d_attn[:])
            nc.sync.dma_start(out=sb_ones[:], in_=d_ones[:])
            nc.sync.dma_start(out=sb_wfold[:], in_=d_wfold[:])

            outT = [p_outT.tile([128, T], F32R, tag=f"outT{j}", name=f"outT{j}") for j in range(NF)]

            # ---- projection phase (pools closed afterwards) ----
            with tc.tile_pool(name="xw", bufs=1) as p_xw, \
                 tc.tile_pool(name="pps", bufs=3, space="PSUM") as p_pps, \
                 tc.tile_pool(name="ppv", bufs=2, space="PSUM") as p_ppv:
                sb_x = p_xw.tile([128, NE * T], F32R)
                sb_wq = p_xw.tile([128, NE * F], F32R)
                sb_wk = p_xw.tile([128, NE * F], F32R)
                sb_wv = p_xw.tile([128, NE * F], F32R)
                nc.sync.dma_start(out=sb_x[:], in_=d_xT[:])
                nc.sync.dma_start(out=sb_wq[:], in_=d_wq[:])
                nc.sync.dma_start(out=sb_wk[:], in_=d_wk[:])
                nc.sync.dma_start(out=sb_wv[:], in_=d_wv[:])
                sb_q = _proj_transposed(nc, p_pps, p_qt, sb_x, sb_wq, sb_qb, "q")
                sb_k = _proj_transposed(nc, p_pps, p_kt, sb_x, sb_wk, sb_kb, "k")
                sb_v = _proj_v(nc, p_ppv, p_v, sb_x, sb_wv)

            # ---- attention maps (branch on core id) ----
            with tc.If(pid < 4) as cmp:
                _attention(nc, tc, sb_q, sb_k, sb_v, sb_attn,
                           sb_ones, outT,
                           maps=[([m], [(m, m)]) for m in range(3)],
                           t_win=1024, scale=SCALE_REG)
            with cmp.Else():
                _attention(nc, tc, sb_q, sb_k, sb_v, sb_attn,
                           sb_ones, outT,
                           maps=[([0, 1, 2], [(0, 0), (1, 1), (2, 2)])],
                           t_win=512, scale=SCALE_CUL, cmask=d_cmask, psc_bufs=3)

            # ---- fold: out[t, :] = sum_c outT[c].T @ wfold[c] ----
            with tc.tile_pool(name="pf", bufs=2, space="PSUM") as p_pf, \
                 tc.tile_pool(name="fin", bufs=3) as p_fin:
                for tt in range(NT):
                    ps_f = p_pf.tile([128, E], F32, tag="pf")
                    for c in range(NF):
                        for e0, e1 in ((0, 512), (512, 768)):
                            nc.tensor.matmul(
                                ps_f[:, e0:e1],
                                lhsT=outT[c][:, tt * 128:(tt + 1) * 128],
                                rhs=sb_wfold[:, c * E + e0: c * E + e1],
                                start=(c == 0), stop=(c == NF - 1))
                    fin = p_fin.tile([128, E], F32, tag="fin")
                    nc.scalar.activation(fin[:], ps_f[:], AF.Copy, bias=0.0, scale=1.0)
                    nc.sync.dma_start(out=d_out[tt * 128:(tt + 1) * 128, :], in_=fin[:])
    nc.compile()
    return nc


def _get_nc():
    global _NC_CACHE
    if _NC_CACHE is None:
        _NC_CACHE = _build_nc()
    return _NC_CACHE


def _chunked_T(a):
    """[E, T]-style [768, X] -> [128, 6*X] with e-chunk-major free layout."""
    e, x = a.shape
    return np.ascontiguousarray(
        a.reshape(e // 128, 128, x).transpose(1, 0, 2).reshape(128, (e // 128) * x))


def kernel(hidden_states, cultural_mask, attention_mask,
           rq_w, rk_w, rv_w, ro_w, cq_w, ck_w, cv_w, co_w,
           rq_b, rk_b, rv_b, ro_b, cq_b, ck_b, cv_b, co_b,
           r_cb, c_cb, out_w, out_b):
    hidden_states = np.asarray(hidden_states)
    nc = _get_nc()
    Wo1 = np.asarray(out_w[:E], np.float64)
    Wo2 = np.asarray(out_w[E:], np.float64)
    wfold_reg = (np.asarray(ro_w, np.float64) @ Wo1)
    wfold_cul = (np.asarray(co_w, np.float64) @ Wo2)
    r_cb_flat = np.asarray(r_cb, np.float64).reshape(-1)  # [NH_REG*128] = [768]
    c_cb_flat = np.asarray(c_cb, np.float64).reshape(-1)  # [NH_CUL*384] = [768]
    qb_reg_full = np.asarray(rq_b, np.float64) + r_cb_flat
    qb_cul_full = np.asarray(cq_b, np.float64) + c_cb_flat

    ones = np.ones((128, 1), np.float32)
    zeros_cm = np.zeros((T, T), np.float32)
    in_maps = []
    for core in range(8):
        if core < 4:
            b, h0 = core // 2, (core % 2) * 3
            cols = slice(h0 * 128, h0 * 128 + F)
            wq_l, wk_l, wv_l = rq_w[:, cols], rk_w[:, cols], rv_w[:, cols]
            qb_l = qb_reg_full[cols]
            kb_l = np.asarray(rk_b, np.float64)[cols]
            wfold_l = wfold_reg[cols]
            cm_l = zeros_cm
            scale_inv = 1.0
        else:
            b, h = (core - 4) // 2, (core - 4) % 2
            cols = slice(h * F, (h + 1) * F)
            wq_l, wk_l, wv_l = cq_w[:, cols], ck_w[:, cols], cv_w[:, cols]
            qb_l = qb_cul_full[cols]
            kb_l = np.asarray(ck_b, np.float64)[cols]
            wfold_l = wfold_cul[cols]
            cm_l = np.ascontiguousarray(
                np.asarray(cultural_mask[b], np.float32).T) * np.float32(1.0 / SCALE_CUL)
            scale_inv = 1.0
        xT = np.asarray(hidden_states[b], np.float32).T  # [768, 2048]
        in_maps.append({
            "xT": _chunked_T(np.ascontiguousarray(xT)),
            "wq": _chunked_T(np.asarray(wq_l, np.float32)),
            "wk": _chunked_T(np.asarray(wk_l, np.float32)),
            "wv": _chunked_T(np.asarray(wv_l, np.float32)),
            "qb": np.ascontiguousarray(np.asarray(qb_l, np.float32).reshape(NF, 128).T),
            "kb": np.ascontiguousarray(np.asarray(kb_l, np.float32).reshape(NF, 128).T),
            "attn": np.ascontiguousarray(
                np.asarray(attention_mask[b, 0, 0, :], np.float32).reshape(NT, 128).T),
            "wfold": _chunked_T(np.asarray(wfold_l, np.float32)),
            "ones": ones,
            "cmask": cm_l,
            "out": np.zeros((T, E), np.float32),
        })
    for im in in_maps:
        im.pop("out")

    res = run_bass_kernel_spmd(nc, in_maps, list(range(8))).results

    bias_total = (np.asarray(out_b, np.float64)
                  + np.asarray(ro_b, np.float64) @ Wo1
                  + np.asarray(co_b, np.float64) @ Wo2
                  + np.asarray(rv_b, np.float64) @ np.asarray(ro_w, np.float64) @ Wo1
                  + np.asarray(cv_b, np.float64) @ np.asarray(co_w, np.float64) @ Wo2)
    out = np.empty((B, T, E), np.float32)
    for b in range(B):
        acc = (res[2 * b]["out"].astype(np.float64)
               + res[2 * b + 1]["out"].astype(np.float64)
               + res[4 + 2 * b]["out"].astype(np.float64)
               + res[5 + 2 * b]["out"].astype(np.float64)
               + bias_total)
        out[b] = acc.astype(np.float32)
    return out



# revision 4
# speedup vs baseline: 1.3599x; 1.3599x over previous
"""Trainium2 Bass kernel for nn_MultiHeadCulturalAttention.

Sharding (8 cores, SPMD single program with a partition-id branch):
  cores 0-3: "regular" branch — (batch b = core//2), 3 heads of hd=128 each
  cores 4-7: "cultural" branch — (batch b = (core-4)//2), 1 head of hd=384

All streaming operands are float16 (PE runs 16-bit at full rate with fast
weight loads; DVE gets 2-4x modes); accumulation is fp32 in PSUM except
where noted. Softmax denominators come from DVE-accumulated exp tiles
reduced by a single ones[128,128] broadcast-matmul per t-window, then a
DVE reciprocal + multiply (no gpsimd, no DMA round-trips).

Per core: Q^T/K^T/V projections for its 384 feature columns, attention
with transposed scores (s on partitions), folded output projection
wfold = branch_out_w @ out_w_half, DMA of a [2048, 768] fp32 partial.
Host sums 4 partials per batch and adds a constant bias vector.
"""
import numpy as np

import concourse.bass as bass
import concourse.mybir as mybir
from concourse import bacc
from concourse.tile import TileContext
from concourse.bass_utils import run_bass_kernel_spmd

F32 = mybir.dt.float32
F16 = mybir.dt.float16
AF = mybir.ActivationFunctionType
ALU = mybir.AluOpType

B, T, E = 2, 2048, 768
NE = E // 128            # 6 e-chunks
NT = T // 128            # 16 tiles along seq
F = 384                  # per-core projection width (3 reg heads / 1 cul head)
NF = F // 128            # 3 f-tiles
SCALE_REG = float(128 ** -0.5)
SCALE_CUL = float(384 ** -0.5)

_NC_CACHE = None


def _proj_transposed(nc, pool_ps, pool_out, sb_x, sb_w, sb_bias, tag):
    """Q^T/K^T: out[f,t] = sum_e w[e,f] x^T[e,t], evacuated on the scalar
    engine with per-partition bias add. Returns 3 SBUF tiles [128, T] f16."""
    outs = []
    for f in range(NF):
        tiles_th = []
        for th in range(2):  # halves of T -> psum [128, 1024]
            ps = pool_ps.tile([128, 1024], F32, tag="pp")
            for e in range(NE):
                for tq in range(2):
                    nc.tensor.matmul(
                        ps[:, tq * 512:(tq + 1) * 512],
                        lhsT=sb_w[:, e * F + f * 128: e * F + (f + 1) * 128],
                        rhs=sb_x[:, e * T + th * 1024 + tq * 512: e * T + th * 1024 + (tq + 1) * 512],
                        start=(e == 0), stop=(e == NE - 1))
            tiles_th.append(ps)
        sb_o = pool_out.tile([128, T], F16, tag=f"{tag}{f}")
        for th in range(2):
            nc.scalar.activation(sb_o[:, th * 1024:(th + 1) * 1024], tiles_th[th][:],
                                 AF.Identity, bias=sb_bias[:, f:f + 1], scale=1.0)
        outs.append(sb_o)
    return outs


def _proj_v(nc, pool_ps, pool_out, sb_x, sb_wv):
    """V: out[s,d] = sum_e x^T[e,s] wv[e,d]. Returns 16 tiles [128, F] f16,
    evacuated on DVE."""
    outs = []
    for s in range(NT):
        ps = pool_ps.tile([128, F], F32, tag="ppv")
        for e in range(NE):
            nc.tensor.matmul(
                ps[:], lhsT=sb_x[:, e * T + s * 128: e * T + (s + 1) * 128],
                rhs=sb_wv[:, e * F:(e + 1) * F],
                start=(e == 0), stop=(e == NE - 1))
        sb_v = pool_out.tile([128, F], F16, tag=f"v{s}")
        nc.vector.tensor_copy(sb_v[:], ps[:])
        outs.append(sb_v)
    return outs


GRP = 8  # exp-tile accumulation group size (fp16 rounding ~ sqrt(GRP) ulp)


def _attention(nc, tc, sb_q, sb_k, sb_v, sb_attn, sb_ones, outT,
               maps, t_win, scale, em=None, po_bufs=2):
    """maps: list of (score_chunk_list, av_list); av_list = [(v_chunk, out_idx)].
    scores^T per s-block -> exp (+mask mult) -> DVE group-acc -> AV matmuls;
    denominator = ones128 @ group-accs (psum), DVE reciprocal, DVE multiply."""
    n_ti = T // t_win
    n_tq = t_win // 512
    n_grp = NT // GRP
    from contextlib import ExitStack
    with ExitStack() as stk:
        p_psc = stk.enter_context(tc.tile_pool(name="psc", bufs=2, space="PSUM"))
        p_po = stk.enter_context(tc.tile_pool(name="po", bufs=po_bufs, space="PSUM"))
        p_wt = stk.enter_context(tc.tile_pool(name="wt", bufs=3))
        p_acc = stk.enter_context(tc.tile_pool(name="acc", bufs=2))
        p_rec = stk.enter_context(tc.tile_pool(name="rec", bufs=2))
        p_em = stk.enter_context(tc.tile_pool(name="em", bufs=3)) if em is not None else None

        for chunks, avs in maps:
            for ti in range(n_ti):
                t0 = ti * t_win
                ps_o = [p_po.tile([128, t_win], F32, tag=f"po{j}", name=f"ps_o{j}")
                        for j in range(len(avs))]
                accs = [p_acc.tile([128, t_win], F16, tag=f"acc{g}", name=f"acc{g}")
                        for g in range(n_grp)]
                for s in range(NT):
                    ps_sc = p_psc.tile([128, t_win], F32, tag="psc")
                    for ci, c in enumerate(chunks):
                        for tq in range(n_tq):
                            nc.tensor.matmul(
                                ps_sc[:, tq * 512:(tq + 1) * 512],
                                lhsT=sb_k[c][:, s * 128:(s + 1) * 128],
                                rhs=sb_q[c][:, t0 + tq * 512: t0 + (tq + 1) * 512],
                                start=(ci == 0), stop=(ci == len(chunks) - 1))
                    wt = p_wt.tile([128, t_win], F16, tag="wt")
                    if em is not None:
                        wt0 = p_wt.tile([128, t_win], F16, tag="wt0")
                        nc.scalar.activation(wt0[:], ps_sc[:], AF.Exp,
                                             bias=0.0, scale=scale)
                        cm = p_em.tile([128, t_win], F16, tag="em")
                        nc.sync.dma_start(out=cm[:], in_=em[s, :, t0:t0 + t_win])
                        nc.vector.tensor_tensor(wt[:], wt0[:], cm[:], ALU.mult)
                    else:
                        nc.scalar.activation(wt[:], ps_sc[:], AF.Exp,
                                             bias=sb_attn[:, s:s + 1], scale=scale)
                    g = s // GRP
                    if s % GRP == 0:
                        nc.vector.tensor_copy(accs[g][:], wt[:])
                    else:
                        nc.vector.tensor_tensor(accs[g][:], accs[g][:], wt[:], ALU.add)
                    for j, (vc, _oidx) in enumerate(avs):
                        for tq in range(n_tq):
                            nc.tensor.matmul(
                                ps_o[j][:, tq * 512:(tq + 1) * 512],
                                lhsT=sb_v[s][:, vc * 128:(vc + 1) * 128],
                                rhs=wt[:, tq * 512:(tq + 1) * 512],
                                start=(s == 0), stop=(s == NT - 1))
                # denominator: broadcast column-sums of group-accs to all
                # 128 partitions with a ones[128,128] stationary matmul
                ps_den = p_psc.tile([128, t_win], F32, tag="psc", name="ps_den")
                for g in range(n_grp):
                    for tq in range(n_tq):
                        nc.tensor.matmul(
                            ps_den[:, tq * 512:(tq + 1) * 512],
                            lhsT=sb_ones[:, 0:128],
                            rhs=accs[g][:, tq * 512:(tq + 1) * 512],
                            start=(g == 0), stop=(g == n_grp - 1))
                rec = p_rec.tile([128, t_win], F32, tag="rec")
                nc.vector.reciprocal(rec[:], ps_den[:])
                for j, (_vc, oidx) in enumerate(avs):
                    nc.vector.tensor_tensor(outT[oidx][:, t0:t0 + t_win],
                                            ps_o[j][:], rec[:], ALU.mult)


def _build_nc():
    nc = bacc.Bacc()
    d_xT = [nc.declare_dram_parameter(f"xT{e}", [128, T], F16, isOutput=False)
            for e in range(NE)]
    d_wq = nc.declare_dram_parameter("wq", [128, NE * F], F16, isOutput=False)
    d_wk = nc.declare_dram_parameter("wk", [128, NE * F], F16, isOutput=False)
    d_wv = nc.declare_dram_parameter("wv", [128, NE * F], F16, isOutput=False)
    d_qb = nc.declare_dram_parameter("qb", [128, NF], F32, isOutput=False)
    d_kb = nc.declare_dram_parameter("kb", [128, NF], F32, isOutput=False)
    d_attn = nc.declare_dram_parameter("attn", [128, NT], F32, isOutput=False)
    d_wfold = nc.declare_dram_parameter("wfold", [128, NF * E], F16, isOutput=False)
    d_ones = nc.declare_dram_parameter("ones", [128, 512], F16, isOutput=False)
    d_em = nc.declare_dram_parameter("em", [NT, 128, T], F16, isOutput=False)
    d_out = nc.declare_dram_parameter("out", [T, E], F32, isOutput=True)

    with TileContext(nc) as tc:
        pid = nc.partition_id()
        from contextlib import ExitStack
        with ExitStack() as stk:
            # ---- persistent pools ----
            p_small = stk.enter_context(tc.tile_pool(name="small", bufs=1))
            p_qt = stk.enter_context(tc.tile_pool(name="qt", bufs=1))
            p_kt = stk.enter_context(tc.tile_pool(name="kt", bufs=1))
            p_v = stk.enter_context(tc.tile_pool(name="vp", bufs=1))
            p_outT = stk.enter_context(tc.tile_pool(name="outT", bufs=1))
            p_wfold = stk.enter_context(tc.tile_pool(name="wfp", bufs=1))

            sb_ones = p_small.tile([128, 512], F16)
            sb_qb = p_small.tile([128, NF], F32)
            sb_kb = p_small.tile([128, NF], F32)
            sb_attn = p_small.tile([128, NT], F32)
            sb_wfold = p_wfold.tile([128, NF * E], F16)
            nc.sync.dma_start(out=sb_ones[:], in_=d_ones[:])
            nc.sync.dma_start(out=sb_qb[:], in_=d_qb[:])
            nc.sync.dma_start(out=sb_kb[:], in_=d_kb[:])
            nc.sync.dma_start(out=sb_attn[:], in_=d_attn[:])

            outT = [p_outT.tile([128, T], F16, tag=f"outT{j}", name=f"outT{j}") for j in range(NF)]

            # ---- warmup: keep PE busy while inputs stream (HAM un-throttle) ----
            with tc.tile_pool(name="wu", bufs=1, space="PSUM") as p_wu:
                ps_wu = p_wu.tile([128, 512], F32)
                for _ in range(10):
                    nc.tensor.matmul(ps_wu[:], lhsT=sb_ones[:, 0:128],
                                     rhs=sb_ones[:], start=True, stop=True)

            # ---- projection phase ----
            with tc.tile_pool(name="xw", bufs=1) as p_xw, \
                 tc.tile_pool(name="pps", bufs=2, space="PSUM") as p_pps, \
                 tc.tile_pool(name="ppv", bufs=2, space="PSUM") as p_ppv:
                sb_wq = p_xw.tile([128, NE * F], F16, tag="wq")
                sb_wk = p_xw.tile([128, NE * F], F16, tag="wk")
                sb_wv = p_xw.tile([128, NE * F], F16, tag="wv")
                sb_x = p_xw.tile([128, NE * T], F16, tag="x")
                nc.sync.dma_start(out=sb_wq[:], in_=d_wq[:])
                nc.sync.dma_start(out=sb_wk[:], in_=d_wk[:])
                nc.sync.dma_start(out=sb_wv[:], in_=d_wv[:])
                for e in range(NE):
                    nc.sync.dma_start(out=sb_x[:, e * T:(e + 1) * T], in_=d_xT[e][:])
                sb_q = _proj_transposed(nc, p_pps, p_qt, sb_x, sb_wq, sb_qb, "q")
                sb_k = _proj_transposed(nc, p_pps, p_kt, sb_x, sb_wk, sb_kb, "k")
                sb_v = _proj_v(nc, p_ppv, p_v, sb_x, sb_wv)

            # ---- attention maps (branch on core id) ----
            with tc.If(pid < 4) as cmp:
                _attention(nc, tc, sb_q, sb_k, sb_v, sb_attn, sb_ones, outT,
                           maps=[([m], [(m, m)]) for m in range(3)],
                           t_win=1024, scale=SCALE_REG, po_bufs=2)
            with cmp.Else():
                _attention(nc, tc, sb_q, sb_k, sb_v, sb_attn, sb_ones, outT,
                           maps=[([0, 1, 2], [(0, 0), (1, 1), (2, 2)])],
                           t_win=512, scale=SCALE_CUL, em=d_em,
                           po_bufs=2)

            # ---- fold: out[t, :] = sum_c outT[c].T @ wfold[c] ----
            nc.sync.dma_start(out=sb_wfold[:], in_=d_wfold[:])
            with tc.tile_pool(name="pf", bufs=2, space="PSUM") as p_pf, \
                 tc.tile_pool(name="fin", bufs=3) as p_fin:
                for tt in range(NT):
                    ps_f = p_pf.tile([128, E], F32, tag="pf")
                    for c in range(NF):
                        for e0, e1 in ((0, 512), (512, 768)):
                            nc.tensor.matmul(
                                ps_f[:, e0:e1],
                                lhsT=outT[c][:, tt * 128:(tt + 1) * 128],
                                rhs=sb_wfold[:, c * E + e0: c * E + e1],
                                start=(c == 0), stop=(c == NF - 1))
                    fin = p_fin.tile([128, E], F32, tag="fin")
                    nc.scalar.activation(fin[:], ps_f[:], AF.Copy, bias=0.0, scale=1.0)
                    nc.sync.dma_start(out=d_out[tt * 128:(tt + 1) * 128, :], in_=fin[:])
    nc.compile()
    return nc


def _get_nc():
    global _NC_CACHE
    if _NC_CACHE is None:
        _NC_CACHE = _build_nc()
    return _NC_CACHE


def _chunked_T(a, dt=np.float16):
    """[E, X]-style [768, X] -> [128, 6*X] with e-chunk-major free layout."""
    e, x = a.shape
    return np.ascontiguousarray(
        a.reshape(e // 128, 128, x).transpose(1, 0, 2).reshape(128, (e // 128) * x)
    ).astype(dt)


def kernel(hidden_states, cultural_mask, attention_mask,
           rq_w, rk_w, rv_w, ro_w, cq_w, ck_w, cv_w, co_w,
           rq_b, rk_b, rv_b, ro_b, cq_b, ck_b, cv_b, co_b,
           r_cb, c_cb, out_w, out_b):
    hidden_states = np.asarray(hidden_states)
    nc = _get_nc()
    Wo1 = np.asarray(out_w[:E], np.float64)
    Wo2 = np.asarray(out_w[E:], np.float64)
    wfold_reg = (np.asarray(ro_w, np.float64) @ Wo1)
    wfold_cul = (np.asarray(co_w, np.float64) @ Wo2)
    r_cb_flat = np.asarray(r_cb, np.float64).reshape(-1)  # [768]
    c_cb_flat = np.asarray(c_cb, np.float64).reshape(-1)  # [768]
    qb_reg_full = np.asarray(rq_b, np.float64) + r_cb_flat
    qb_cul_full = np.asarray(cq_b, np.float64) + c_cb_flat

    ones = np.ones((128, 512), np.float16)
    em_zero = np.zeros((NT, 128, T), np.float16)
    attn_np = np.asarray(attention_mask, np.float32)
    in_maps = []
    for core in range(8):
        if core < 4:
            b, h0 = core // 2, (core % 2) * 3
            cols = slice(h0 * 128, h0 * 128 + F)
            wq_l, wk_l, wv_l = rq_w[:, cols], rk_w[:, cols], rv_w[:, cols]
            qb_l = qb_reg_full[cols]
            kb_l = np.asarray(rk_b, np.float64)[cols]
            wfold_l = wfold_reg[cols]
            em_l = em_zero
        else:
            b, h = (core - 4) // 2, (core - 4) % 2
            cols = slice(h * F, (h + 1) * F)
            wq_l, wk_l, wv_l = cq_w[:, cols], ck_w[:, cols], cv_w[:, cols]
            qb_l = qb_cul_full[cols]
            kb_l = np.asarray(ck_b, np.float64)[cols]
            wfold_l = wfold_cul[cols]
            # exp(cultural_mask^T + attention_mask[s]) as [s_chunk, p, t]
            em_f = np.exp(np.asarray(cultural_mask[b], np.float64).T
                          + attn_np[b, 0, 0, :][:, None])
            em_l = np.ascontiguousarray(
                em_f.reshape(NT, 128, T)).astype(np.float16)
        xT = np.asarray(hidden_states[b], np.float32).T  # [768, 2048]
        xT_c = _chunked_T(np.ascontiguousarray(xT))
        im = {
            "wq": _chunked_T(np.asarray(wq_l, np.float32)),
            "wk": _chunked_T(np.asarray(wk_l, np.float32)),
            "wv": _chunked_T(np.asarray(wv_l, np.float32)),
            "qb": np.ascontiguousarray(np.asarray(qb_l, np.float32).reshape(NF, 128).T),
            "kb": np.ascontiguousarray(np.asarray(kb_l, np.float32).reshape(NF, 128).T),
            "attn": np.ascontiguousarray(attn_np[b, 0, 0, :].reshape(NT, 128).T),
            "wfold": _chunked_T(np.asarray(wfold_l, np.float32)),
            "ones": ones,
            "em": em_l,
        }
        for e in range(NE):
            im[f"xT{e}"] = np.ascontiguousarray(xT_c[:, e * T:(e + 1) * T])
        in_maps.append(im)

    res = run_bass_kernel_spmd(nc, in_maps, list(range(8))).results

    bias_total = (np.asarray(out_b, np.float64)
                  + np.asarray(ro_b, np.float64) @ Wo1
                  + np.asarray(co_b, np.float64) @ Wo2
                  + np.asarray(rv_b, np.float64) @ np.asarray(ro_w, np.float64) @ Wo1
                  + np.asarray(cv_b, np.float64) @ np.asarray(co_w, np.float64) @ Wo2)
    out = np.empty((B, T, E), np.float32)
    for b in range(B):
        acc = (res[2 * b]["out"].astype(np.float64)
               + res[2 * b + 1]["out"].astype(np.float64)
               + res[4 + 2 * b]["out"].astype(np.float64)
               + res[5 + 2 * b]["out"].astype(np.float64)
               + bias_total)
        out[b] = acc.astype(np.float32)
    return out


# revision 6
# speedup vs baseline: 1.5641x; 1.1502x over previous
"""Trainium2 Bass kernel for nn_MultiHeadCulturalAttention.

Sharding (8 cores, SPMD single program with a partition-id branch):
  cores 0-3: "regular" branch — (batch b = core//2), 3 heads of hd=128 each
  cores 4-7: "cultural" branch — (batch b = (core-4)//2), 1 head of hd=384

All streaming operands are float16 (PE runs 16-bit at full rate with fast
weight loads; DVE gets 2-4x modes); accumulation is fp32 in PSUM.
Softmax denominators come from DVE-accumulated exp tiles reduced by a
ones[128,128] broadcast-matmul per t-window, then a fast-approx DVE
reciprocal + multiply (no gpsimd, no DMA round-trips).

Per core: Q^T/K^T/V projections for its 384 feature columns (T-quarter-
major loop order so the PE starts as soon as the first x quarter lands),
attention with transposed scores (s on partitions), folded output
projection wfold = branch_out_w @ out_w_half with the fold matmuls
interleaved into the last attention map, DMA of a [2048, 768] fp32
partial. Host sums 4 partials per batch and adds a constant bias vector.
"""
import numpy as np

import concourse.bass as bass
import concourse.mybir as mybir
from concourse import bacc
from concourse.tile import TileContext
from concourse.bass_utils import run_bass_kernel_spmd

F32 = mybir.dt.float32
F16 = mybir.dt.float16
AF = mybir.ActivationFunctionType
ALU = mybir.AluOpType

B, T, E = 2, 2048, 768
NE = E // 128            # 6 e-chunks
NT = T // 128            # 16 tiles along seq
F = 384                  # per-core projection width (3 reg heads / 1 cul head)
NF = F // 128            # 3 f-tiles
SCALE_REG = float(128 ** -0.5)
SCALE_CUL = float(384 ** -0.5)
GRP = 8                  # exp-tile accumulation group size

_NC_CACHE = None


def _proj_qk(nc, pool_ps, p_qt, p_kt, sb_x, sb_wq, sb_wk, sb_qb, sb_kb):
    """Q^T/K^T [f,t] = sum_e w[e,f] x^T[e,t], T-quarter-major so compute
    starts once the first x quarter has landed. Scalar-engine evacuation
    with per-partition bias add."""
    sb_q = [p_qt.tile([128, T], F16, tag=f"q{f}", name=f"q{f}") for f in range(NF)]
    sb_k = [p_kt.tile([128, T], F16, tag=f"k{f}", name=f"k{f}") for f in range(NF)]
    for tq in range(4):
        c0 = tq * 512
        for sb_w, sb_b, outs in ((sb_wq, sb_qb, sb_q), (sb_wk, sb_kb, sb_k)):
            for f in range(NF):
                ps = pool_ps.tile([128, 512], F32, tag="pp", name="ps_p")
                for e in range(NE):
                    nc.tensor.matmul(
                        ps[:],
                        lhsT=sb_w[:, e * F + f * 128: e * F + (f + 1) * 128],
                        rhs=sb_x[:, e * T + c0: e * T + c0 + 512],
                        start=(e == 0), stop=(e == NE - 1))
                nc.scalar.activation(outs[f][:, c0:c0 + 512], ps[:],
                                     AF.Identity, bias=sb_b[:, f:f + 1], scale=1.0)
    return sb_q, sb_k


def _proj_v(nc, pool_ps, pool_out, sb_x, sb_wv):
    """V: out[s,d] = sum_e x^T[e,s] wv[e,d]. 16 tiles [128, F] f16, DVE evac."""
    outs = []
    for s in range(NT):
        ps = pool_ps.tile([128, F], F32, tag="ppv", name="ps_v")
        for e in range(NE):
            nc.tensor.matmul(
                ps[:], lhsT=sb_x[:, e * T + s * 128: e * T + (s + 1) * 128],
                rhs=sb_wv[:, e * F:(e + 1) * F],
                start=(e == 0), stop=(e == NE - 1))
        sb_v = pool_out.tile([128, F], F16, tag=f"v{s}", name=f"v{s}")
        nc.vector.tensor_copy(sb_v[:], ps[:])
        outs.append(sb_v)
    return outs


def _attention(nc, tc, sb_q, sb_k, sb_v, sb_attn, sb_ones, outT,
               maps, scale, em=None, fold_cb=None):
    """maps: list of (score_chunks, av_list, t_win); av_list = [(v_chunk,
    out_idx)]. scores^T per s-block -> exp (+mask mult) -> DVE group-acc ->
    AV matmuls; denominator = ones128 @ group-accs (psum), fast reciprocal,
    DVE multiply. fold_cb(t0, t_win), if given, runs after the final map's
    per-window normalize (fold matmuls fill the exp-paced PE idle)."""
    n_grp = NT // GRP
    from contextlib import ExitStack
    with ExitStack() as stk:
        p_wt = stk.enter_context(tc.tile_pool(name="wt", bufs=3))
        p_acc = stk.enter_context(tc.tile_pool(name="acc", bufs=2))
        p_rec = stk.enter_context(tc.tile_pool(name="rec", bufs=2))
        p_em = stk.enter_context(tc.tile_pool(name="em", bufs=3)) if em is not None else None

        for mi, (chunks, avs, t_win) in enumerate(maps):
            n_tq = t_win // 512
            last_map = mi == len(maps) - 1
            mstk = ExitStack()
            p_psc = mstk.enter_context(
                tc.tile_pool(name=f"psc{mi}", bufs=2, space="PSUM"))
            p_po = mstk.enter_context(
                tc.tile_pool(name=f"po{mi}", bufs=2, space="PSUM"))
            if last_map and fold_cb is not None:
                fold_cb = fold_cb(mstk)  # enter fold pools now
            for ti in range(T // t_win):
                t0 = ti * t_win
                ps_o = [p_po.tile([128, t_win], F32, tag=f"po{j}", name=f"ps_o{j}")
                        for j in range(len(avs))]
                accs = [p_acc.tile([128, t_win], F16, tag=f"acc{g}", name=f"acc{g}")
                        for g in range(n_grp)]
                for s in range(NT):
                    ps_sc = p_psc.tile([128, t_win], F32, tag="psc", name="ps_sc")
                    for ci, c in enumerate(chunks):
                        for tq in range(n_tq):
                            nc.tensor.matmul(
                                ps_sc[:, tq * 512:(tq + 1) * 512],
                                lhsT=sb_k[c][:, s * 128:(s + 1) * 128],
                                rhs=sb_q[c][:, t0 + tq * 512: t0 + (tq + 1) * 512],
                                start=(ci == 0), stop=(ci == len(chunks) - 1))
                    wt = p_wt.tile([128, t_win], F16, tag="wt", name="wt")
                    if em is not None:
                        wt0 = p_wt.tile([128, t_win], F16, tag="wt0", name="wt0")
                        nc.scalar.activation(wt0[:], ps_sc[:], AF.Exp,
                                             bias=0.0, scale=scale)
                        cm = p_em.tile([128, t_win], F16, tag="em", name="cm")
                        nc.sync.dma_start(out=cm[:], in_=em[s, :, t0:t0 + t_win])
                        nc.vector.tensor_tensor(wt[:], wt0[:], cm[:], ALU.mult)
                    else:
                        nc.scalar.activation(wt[:], ps_sc[:], AF.Exp,
                                             bias=sb_attn[:, s:s + 1], scale=scale)
                    g = s // GRP
                    if s % GRP == 0:
                        nc.vector.tensor_copy(accs[g][:], wt[:])
                    else:
                        nc.vector.tensor_tensor(accs[g][:], accs[g][:], wt[:], ALU.add)
                    for j, (vc, _oidx) in enumerate(avs):
                        for tq in range(n_tq):
                            nc.tensor.matmul(
                                ps_o[j][:, tq * 512:(tq + 1) * 512],
                                lhsT=sb_v[s][:, vc * 128:(vc + 1) * 128],
                                rhs=wt[:, tq * 512:(tq + 1) * 512],
                                start=(s == 0), stop=(s == NT - 1))
                # denominator broadcast to all partitions via ones[128,128]
                ps_den = p_psc.tile([128, t_win], F32, tag="psc", name="ps_den")
                for g in range(n_grp):
                    for tq in range(n_tq):
                        nc.tensor.matmul(
                            ps_den[:, tq * 512:(tq + 1) * 512],
                            lhsT=sb_ones[:, 0:128],
                            rhs=accs[g][:, tq * 512:(tq + 1) * 512],
                            start=(g == 0), stop=(g == n_grp - 1))
                rec = p_rec.tile([128, t_win], F32, tag="rec", name="rec")
                nc.vector.reciprocal_approx_fast(out=rec[:], in_=ps_den[:])
                for j, (_vc, oidx) in enumerate(avs):
                    nc.vector.tensor_tensor(outT[oidx][:, t0:t0 + t_win],
                                            ps_o[j][:], rec[:], ALU.mult)
                if last_map and fold_cb is not None:
                    fold_cb(t0, t_win)
            mstk.close()


def _build_nc():
    nc = bacc.Bacc()
    d_xT = [nc.declare_dram_parameter(f"xT{e}", [128, T], F16, isOutput=False)
            for e in range(NE)]
    d_wq = nc.declare_dram_parameter("wq", [128, NE * F], F16, isOutput=False)
    d_wk = nc.declare_dram_parameter("wk", [128, NE * F], F16, isOutput=False)
    d_wv = nc.declare_dram_parameter("wv", [128, NE * F], F16, isOutput=False)
    d_qb = nc.declare_dram_parameter("qb", [128, NF], F32, isOutput=False)
    d_kb = nc.declare_dram_parameter("kb", [128, NF], F32, isOutput=False)
    d_attn = nc.declare_dram_parameter("attn", [128, NT], F32, isOutput=False)
    d_wfold = nc.declare_dram_parameter("wfold", [128, NF * E], F16, isOutput=False)
    d_ones = nc.declare_dram_parameter("ones", [128, 512], F16, isOutput=False)
    d_em = nc.declare_dram_parameter("em", [NT, 128, T], F16, isOutput=False)
    d_out = nc.declare_dram_parameter("out", [T, E], F32, isOutput=True)

    with TileContext(nc) as tc:
        pid = nc.partition_id()
        from contextlib import ExitStack
        with ExitStack() as stk:
            # ---- persistent pools ----
            p_small = stk.enter_context(tc.tile_pool(name="small", bufs=1))
            p_qt = stk.enter_context(tc.tile_pool(name="qt", bufs=1))
            p_kt = stk.enter_context(tc.tile_pool(name="kt", bufs=1))
            p_v = stk.enter_context(tc.tile_pool(name="vp", bufs=1))
            p_outT = stk.enter_context(tc.tile_pool(name="outT", bufs=1))
            p_wfold = stk.enter_context(tc.tile_pool(name="wfp", bufs=1))

            sb_ones = p_small.tile([128, 512], F16)
            sb_qb = p_small.tile([128, NF], F32)
            sb_kb = p_small.tile([128, NF], F32)
            sb_attn = p_small.tile([128, NT], F32)
            sb_wfold = p_wfold.tile([128, NF * E], F16)
            nc.sync.dma_start(out=sb_ones[:], in_=d_ones[:])
            nc.sync.dma_start(out=sb_qb[:], in_=d_qb[:])
            nc.sync.dma_start(out=sb_kb[:], in_=d_kb[:])
            nc.sync.dma_start(out=sb_attn[:], in_=d_attn[:])

            outT = [p_outT.tile([128, T], F16, tag=f"outT{j}", name=f"outT{j}")
                    for j in range(NF)]

            # ---- warmup: PE busy while inputs stream (HAM un-throttle) ----
            with tc.tile_pool(name="wu", bufs=1, space="PSUM") as p_wu:
                ps_wu = p_wu.tile([128, 512], F32)
                for _ in range(10):
                    nc.tensor.matmul(ps_wu[:], lhsT=sb_ones[:, 0:128],
                                     rhs=sb_ones[:], start=True, stop=True)

            # ---- projection phase ----
            with tc.tile_pool(name="xw", bufs=1) as p_xw, \
                 tc.tile_pool(name="pps", bufs=4, space="PSUM") as p_pps, \
                 tc.tile_pool(name="ppv", bufs=2, space="PSUM") as p_ppv:
                sb_wq = p_xw.tile([128, NE * F], F16, tag="wq")
                sb_wk = p_xw.tile([128, NE * F], F16, tag="wk")
                sb_wv = p_xw.tile([128, NE * F], F16, tag="wv")
                sb_x = p_xw.tile([128, NE * T], F16, tag="x")
                nc.sync.dma_start(out=sb_wq[:], in_=d_wq[:])
                nc.sync.dma_start(out=sb_wk[:], in_=d_wk[:])
                for tq in range(4):
                    for e in range(NE):
                        nc.sync.dma_start(
                            out=sb_x[:, e * T + tq * 512: e * T + (tq + 1) * 512],
                            in_=d_xT[e][:, tq * 512:(tq + 1) * 512])
                nc.sync.dma_start(out=sb_wv[:], in_=d_wv[:])
                sb_q, sb_k = _proj_qk(nc, p_pps, p_qt, p_kt, sb_x,
                                      sb_wq, sb_wk, sb_qb, sb_kb)
                sb_v = _proj_v(nc, p_ppv, p_v, sb_x, sb_wv)

            nc.sync.dma_start(out=sb_wfold[:], in_=d_wfold[:])

            # ---- attention + interleaved fold (branch on core id) ----
            def fold_range(p_pf, p_fin, t0, t_win):
                for tt in range(t0 // 128, (t0 + t_win) // 128):
                    ps_f = p_pf.tile([128, E], F32, tag="pf", name="ps_f")
                    for c in range(NF):
                        for e0, e1 in ((0, 512), (512, 768)):
                            nc.tensor.matmul(
                                ps_f[:, e0:e1],
                                lhsT=outT[c][:, tt * 128:(tt + 1) * 128],
                                rhs=sb_wfold[:, c * E + e0: c * E + e1],
                                start=(c == 0), stop=(c == NF - 1))
                    fin = p_fin.tile([128, E], F32, tag="fin", name="fin")
                    nc.scalar.activation(fin[:], ps_f[:], AF.Copy,
                                         bias=0.0, scale=1.0)
                    nc.sync.dma_start(out=d_out[tt * 128:(tt + 1) * 128, :],
                                      in_=fin[:])

            def make_fold_cb(mstk):
                p_pf = mstk.enter_context(
                    tc.tile_pool(name="pfr", bufs=2, space="PSUM"))
                p_fin = mstk.enter_context(tc.tile_pool(name="finr", bufs=3))
                return lambda t0, t_win: fold_range(p_pf, p_fin, t0, t_win)

            with tc.If(pid < 4) as cmp:
                _attention(nc, tc, sb_q, sb_k, sb_v, sb_attn, sb_ones, outT,
                           maps=[([0], [(0, 0)], 1024),
                                 ([1], [(1, 1)], 1024),
                                 ([2], [(2, 2)], 512)],
                           scale=SCALE_REG, fold_cb=make_fold_cb)
            with cmp.Else():
                _attention(nc, tc, sb_q, sb_k, sb_v, sb_attn, sb_ones, outT,
                           maps=[([0, 1, 2], [(0, 0), (1, 1), (2, 2)], 512)],
                           scale=SCALE_CUL, em=d_em)
                with tc.tile_pool(name="pfc", bufs=2, space="PSUM") as p_pfc, \
                     tc.tile_pool(name="finc", bufs=3) as p_finc:
                    for tic in range(4):
                        fold_range(p_pfc, p_finc, tic * 512, 512)
    nc.compile()
    return nc


def _get_nc():
    global _NC_CACHE
    if _NC_CACHE is None:
        _NC_CACHE = _build_nc()
    return _NC_CACHE


def _chunked_T(a, dt=np.float16):
    """[E, X]-style [768, X] -> [128, 6*X] with e-chunk-major free layout."""
    e, x = a.shape
    return np.ascontiguousarray(
        a.reshape(e // 128, 128, x).transpose(1, 0, 2).reshape(128, (e // 128) * x)
    ).astype(dt)


def kernel(hidden_states, cultural_mask, attention_mask,
           rq_w, rk_w, rv_w, ro_w, cq_w, ck_w, cv_w, co_w,
           rq_b, rk_b, rv_b, ro_b, cq_b, ck_b, cv_b, co_b,
           r_cb, c_cb, out_w, out_b):
    hidden_states = np.asarray(hidden_states)
    nc = _get_nc()
    Wo1 = np.asarray(out_w[:E], np.float64)
    Wo2 = np.asarray(out_w[E:], np.float64)
    wfold_reg = (np.asarray(ro_w, np.float64) @ Wo1)
    wfold_cul = (np.asarray(co_w, np.float64) @ Wo2)
    r_cb_flat = np.asarray(r_cb, np.float64).reshape(-1)  # [768]
    c_cb_flat = np.asarray(c_cb, np.float64).reshape(-1)  # [768]
    qb_reg_full = np.asarray(rq_b, np.float64) + r_cb_flat
    qb_cul_full = np.asarray(cq_b, np.float64) + c_cb_flat

    ones = np.ones((128, 512), np.float16)
    em_zero = np.zeros((NT, 128, T), np.float16)
    attn_np = np.asarray(attention_mask, np.float32)
    in_maps = []
    for core in range(8):
        if core < 4:
            b, h0 = core // 2, (core % 2) * 3
            cols = slice(h0 * 128, h0 * 128 + F)
            wq_l, wk_l, wv_l = rq_w[:, cols], rk_w[:, cols], rv_w[:, cols]
            qb_l = qb_reg_full[cols]
            kb_l = np.asarray(rk_b, np.float64)[cols]
            wfold_l = wfold_reg[cols]
            em_l = em_zero
        else:
            b, h = (core - 4) // 2, (core - 4) % 2
            cols = slice(h * F, (h + 1) * F)
            wq_l, wk_l, wv_l = cq_w[:, cols], ck_w[:, cols], cv_w[:, cols]
            qb_l = qb_cul_full[cols]
            kb_l = np.asarray(ck_b, np.float64)[cols]
            wfold_l = wfold_cul[cols]
            # exp(cultural_mask^T + attention_mask[s]) as [s_chunk, p, t]
            em_f = np.exp(np.asarray(cultural_mask[b], np.float64).T
                          + attn_np[b, 0, 0, :][:, None])
            em_l = np.ascontiguousarray(
                em_f.reshape(NT, 128, T)).astype(np.float16)
        xT = np.asarray(hidden_states[b], np.float32).T  # [768, 2048]
        xT_c = _chunked_T(np.ascontiguousarray(xT))
        im = {
            "wq": _chunked_T(np.asarray(wq_l, np.float32)),
            "wk": _chunked_T(np.asarray(wk_l, np.float32)),
            "wv": _chunked_T(np.asarray(wv_l, np.float32)),
            "qb": np.ascontiguousarray(np.asarray(qb_l, np.float32).reshape(NF, 128).T),
            "kb": np.ascontiguousarray(np.asarray(kb_l, np.float32).reshape(NF, 128).T),
            "attn": np.ascontiguousarray(attn_np[b, 0, 0, :].reshape(NT, 128).T),
            "wfold": _chunked_T(np.asarray(wfold_l, np.float32)),
            "ones": ones,
            "em": em_l,
        }
        for e in range(NE):
            im[f"xT{e}"] = np.ascontiguousarray(xT_c[:, e * T:(e + 1) * T])
        in_maps.append(im)

    res = run_bass_kernel_spmd(nc, in_maps, list(range(8))).results

    bias_total = (np.asarray(out_b, np.float64)
                  + np.asarray(ro_b, np.float64) @ Wo1
                  + np.asarray(co_b, np.float64) @ Wo2
                  + np.asarray(rv_b, np.float64) @ np.asarray(ro_w, np.float64) @ Wo1
                  + np.asarray(cv_b, np.float64) @ np.asarray(co_w, np.float64) @ Wo2)
    out = np.empty((B, T, E), np.float32)
    for b in range(B):
        acc = (res[2 * b]["out"].astype(np.float64)
               + res[2 * b + 1]["out"].astype(np.float64)
               + res[4 + 2 * b]["out"].astype(np.float64)
               + res[5 + 2 * b]["out"].astype(np.float64)
               + bias_total)
        out[b] = acc.astype(np.float32)
    return out
